# revision 17
# baseline (speedup 1.0000x reference)
"""Trainium2 Bass kernel for nn_BatchASTEncoder (batched AST / complete-binary-tree
GNN message passing).

Math (per batch column b):
    h[p] = W_c @ encodes[node_ids[p, b]] + b_c                    (1023 tree positions)
    for level d = 8..0:  h[parent] += W_sum @ (h[left] + h[right]) + 2*b_sum
    node_list = relu(h[POSTORDER]);  out2 = max_p node_list

Distribution: data-parallel over batch B=64 across 8 NeuronCores (8 columns
per core); encodes and the tiny weights are replicated.

Per-core dataflow (B_loc=8 batch columns, stage = 2 batch columns, stage-local
column i = bb*1023 + p):
  - h kept feature-major in SBUF: per (e-chunk, stage) tiles [128, 2046].
    Feature-major makes tree pair-sums strided free-dim vector ops and
    matmuls layout-preserving.
  - Software pipeline, one stage in flight per phase: the serial SWDGE gather
    descriptor generation paces the kernel; stage s-1's tree is emitted
    before stage s's PE work so it executes inside stage s's gather window.
  - Gather: one dma_gather per stage (2046 rows + 18 dummies). dma_gather
    indices are int16, so in_ap is based at the table midpoint and indices
    are signed offsets (idx - 32768); trailing dummies keep the ext-isa
    kernel's trailing-negative truncation away from real rows.
  - Matmuls run in float32r (single-pass fp32, ~1e-4 relative error) instead
    of float32 (two-pass); producers of matmul operands write f32r tiles.
  - 2*W_sum_b is folded into the init bias of non-leaf positions.
  - Leaf columns (p >= 511) are final right after init: relu'd and stored
    while the tree for the stage is still pending.
  - Output is written feature-major; the host unshard step transposes to the
    reference row-major [P, B, E] layout and applies the postorder permute.
"""

import numpy as np

DEPTH = 10
P = 2**DEPTH - 1          # 1023
B = 64
E = 256
N_TOTAL = B * P           # 65472
HALF = 32768              # dma_gather int16 base offset
N_CORES = 8
B_LOC = B // N_CORES      # 8
R = B_LOC * P             # 8184 columns per core
NSTAGE = 4
SB = 2                    # batch columns per stage
SW = SB * P               # stage width: 2046
NI = 1040                 # dma_gather indices per call (1 batch column + 17 dummies)
NIB = 9                   # dst j-blocks per call
GATHER = "indirect"       # or "dma_gather"
CPACK = 1156              # packed f32 consts: wc(512) ws(512) bias(4) ident(128)
USE_F32R = True


def _postorder(p, out):
    if p >= P:
        return
    _postorder(2 * p + 1, out)
    _postorder(2 * p + 2, out)
    out.append(p)


_PO = []
_postorder(0, _PO)
POSTORDER = np.array(_PO, dtype=np.int32)

_NC_CACHE = [None]
LAST_RESULT = [None]
TRACE = [False]


def _drain_runs(gi):
    """Per-b column runs for matmul group gi (p in [512gi, 512(gi+1)) cap 1023),
    split at the leaf boundary. Yields (p0, p1, leaf)."""
    runs = []
    p = 512 * gi
    end = min(512 * (gi + 1), P)
    while p < end:
        leaf = p >= 511
        p1 = min(end, P if leaf else 511)
        runs.append((p, p1, leaf))
        p = p1
    return runs


def _build_nc():
    import concourse.bacc as bacc
    import concourse.mybir as mybir
    import concourse.tile as tile
    from concourse import bass  # noqa: F401

    f32 = mybir.dt.float32
    f32m = mybir.dt.float32r if USE_F32R else f32
    i16 = mybir.dt.int16  # noqa
    AF = mybir.ActivationFunctionType
    AX = mybir.AxisListType

    nc = bacc.Bacc("TRN2", target_bir_lowering=False, debug=False)

    encodes = nc.dram_tensor("encodes", [N_TOTAL, E], f32, kind="ExternalInput")
    if GATHER == "dma_gather":
        idx_d = nc.dram_tensor(
            "idx", [128, B_LOC * (NI // 16)], i16, kind="ExternalInput"
        )
    else:
        idx_d = nc.dram_tensor("idx", [128, 64], mybir.dt.int32, kind="ExternalInput")
    cpk_d = nc.dram_tensor("cpack", [128, CPACK], f32, kind="ExternalInput")
    out_nl = nc.dram_tensor("out_nl", [2, NSTAGE, 128, SW], f32, kind="ExternalOutput")
    out_max = nc.dram_tensor("out_max", [128, 2 * B_LOC], f32, kind="ExternalOutput")

    with tile.TileContext(nc) as tc:
        with (
            tc.tile_pool(name="const", bufs=1) as cpool,
            tc.tile_pool(name="h", bufs=1) as hpool,
            tc.tile_pool(name="g", bufs=5) as gpool,
            tc.tile_pool(name="gtg", bufs=3) as gtgpool,
            tc.tile_pool(name="ks", bufs=2) as kspool,
            tc.tile_pool(name="ob", bufs=2) as obpool,
            tc.tile_pool(name="mx", bufs=1) as mxpool,
            tc.tile_pool(name="tp", bufs=2, space="PSUM") as tppool,
            tc.tile_pool(name="hp", bufs=1, space="PSUM") as hppool,
            tc.tile_pool(name="cst", bufs=1, space="PSUM") as cstpool,
        ):
            cpk = cpool.tile([128, CPACK], f32)
            wc = cpool.tile([128, 2 * E], f32m)   # [d-chunk part, dc*256 + e]
            ws = cpool.tile([128, 2 * E], f32m)
            if GATHER == "dma_gather":
                idx = cpool.tile([128, B_LOC * (NI // 16)], i16)
            else:
                idx = cpool.tile([128, 64], mybir.dt.int32)
            hts = [hpool.tile([128, 2 * SW], f32, name=f"hs{s}") for s in range(NSTAGE)]
            mx = mxpool.tile([128, 2 * B_LOC], f32)
            ident = cpk[:, 1028:1156]

            def emit_dummy_mm():
                dmm = dumpool.tile([128, 512], f32, tag="dmm", name="dmm")
                nc.tensor.matmul(
                    out=dmm[:], lhsT=wc[:, 0:128], rhs=wc[:, :],
                    start=True, stop=True, skip_group_check=True,
                )

            def emit_gather(b):
                g = gpool.tile([128, NIB * E], f32, name="g")
                if GATHER == "dma_gather":
                    nc.gpsimd.dma_gather(
                        out_ap=g[:].rearrange("p (j e) -> p j e", e=E),
                        in_ap=encodes[HALF:, :],
                        idxs_ap=idx[:, b * (NI // 16) : (b + 1) * (NI // 16)],
                        num_idxs=NI,
                        num_idxs_reg=NI,
                        elem_size=E,
                        single_packet=False,
                    )
                else:
                    for j in range(8):
                        nc.gpsimd.indirect_dma_start(
                            out=g[:, j * E : (j + 1) * E],
                            out_offset=None,
                            in_=encodes[:, :],
                            in_offset=bass.IndirectOffsetOnAxis(
                                ap=idx[:, b * 8 + j : b * 8 + j + 1], axis=0
                            ),
                        )
                return g

            def emit_init_compute(s, bb, g):
                gtgs = {}
                for gi in range(2):
                    gtg = gtgpool.tile([128, 4 * E], f32m, name="gtg")
                    gtgs[gi] = gtg
                    tpd = [
                        tppool.tile([128, 512], f32, tag="tp", name=f"tpd{dc}")
                        for dc in range(2)
                    ]
                    for k in range(4):
                        j = 4 * gi + k
                        nc.tensor.transpose(
                            out=tpd[0][:, k * 128 : k * 128 + 128],
                            in_=g[:, j * E : j * E + 128],
                            identity=ident,
                        )
                        nc.tensor.transpose(
                            out=tpd[1][:, k * 128 : k * 128 + 128],
                            in_=g[:, j * E + 128 : (j + 1) * E],
                            identity=ident,
                        )
                    nc.vector.tensor_copy(out=gtg[:, 0:512], in_=tpd[0][:])
                    nc.scalar.copy(out=gtg[:, 512:1024], in_=tpd[1][:])
                hp = {
                    (e, gi): hppool.tile(
                        [128, 512], f32, tag=f"hp{e}{gi}", name=f"hp{e}{gi}"
                    )
                    for e in range(2)
                    for gi in range(2)
                }
                # weights-outer: one LDWEIGHTS serves both groups
                for e in range(2):
                    for dc in range(2):
                        lhs = wc[:, 256 * dc + 128 * e : 256 * dc + 128 * e + 128]
                        for gi in range(2):
                            nc.tensor.matmul(
                                out=hp[(e, gi)][:],
                                lhsT=lhs,
                                rhs=gtgs[gi][:, 512 * dc : 512 * dc + 512],
                                start=(dc == 0),
                                stop=(dc == 1),
                            )
                for gi in range(2):
                    for e in range(2):
                        for p0, p1, leaf in _drain_runs(gi):
                            bcol = 1024 + (e if leaf else 2 + e)
                            nc.scalar.activation(
                                out=hts[s][
                                    :, e * SW + bb * P + p0 : e * SW + bb * P + p1
                                ],
                                in_=hp[(e, gi)][:, p0 - 512 * gi : p1 - 512 * gi],
                                func=AF.Identity,
                                bias=cpk[:, bcol : bcol + 1],
                                scale=1.0,
                            )

            def emit_leaf_out(s, bb):
                hv = hts[s][:, :].rearrange("p (e b q) -> p e b q", e=2, b=SB)
                ob = obpool.tile([128, 2 * 512], f32, tag="obl", name="obl")
                obv = ob[:].rearrange("p (e q) -> p e q", e=2)
                nc.scalar.activation(
                    out=obv, in_=hv[:, :, bb, 511:1023], func=AF.Relu
                )
                nc.sync.dma_start(
                    out=out_nl[:, s, :, bb * P + 511 : bb * P + 1023].transpose(
                        [1, 0, 2]
                    ),
                    in_=obv,
                )

            def emit_tree_out(s):
                hv = hts[s][:, :].rearrange("p (e b q) -> p e b q", e=2, b=SB)
                for d in range(DEPTH - 2, -1, -1):
                    p0 = 2**d - 1
                    n = 2**d
                    ks = kspool.tile([128, 2 * SB * n], f32m, tag="ks", name="ks")
                    kid2 = hv[:, :, :, 2 * p0 + 1 : 2 * p0 + 1 + 2 * n].rearrange(
                        "p e b (n two) -> p e b n two", two=2
                    )
                    nc.vector.tensor_add(
                        out=ks[:].rearrange("p (e b n) -> p e b n", e=2, b=SB),
                        in0=kid2[:, :, :, :, 0],
                        in1=kid2[:, :, :, :, 1],
                    )
                    ksv = ks[:].rearrange("p (e bn) -> p e bn", e=2)
                    cs = cstpool.tile([128, 1024], f32, tag="cst", name="cst")
                    for e in range(2):
                        nc.tensor.matmul(
                            out=cs[:, 512 * e : 512 * e + SB * n],
                            lhsT=ws[:, 128 * e : 128 * e + 128],
                            rhs=ksv[:, 0, :],
                            start=True, stop=False,
                        )
                        nc.tensor.matmul(
                            out=cs[:, 512 * e : 512 * e + SB * n],
                            lhsT=ws[:, 256 + 128 * e : 256 + 128 * e + 128],
                            rhs=ksv[:, 1, :],
                            start=False, stop=True,
                        )
                    nc.vector.tensor_add(
                        out=hv[:, :, :, p0 : p0 + n],
                        in0=hv[:, :, :, p0 : p0 + n],
                        in1=cs[:].rearrange("p (e bn) -> p e bn", e=2)[
                            :, :, : SB * n
                        ].rearrange("p e (b n) -> p e b n", b=SB),
                    )
                # non-leaf columns now final: relu + store + stage max
                mxp = mxpool.tile([128, 2 * SB], f32, tag="mxp", name="mxp")
                nc.vector.tensor_reduce(
                    out=mxp[:], in_=hv[:, :, :, :], axis=AX.X,
                    op=mybir.AluOpType.max,
                )
                nc.scalar.activation(
                    out=mx[:, :].rearrange("p (e b) -> p e b", e=2)[
                        :, :, SB * s : SB * (s + 1)
                    ],
                    in_=mxp[:].rearrange("p (e b) -> p e b", e=2),
                    func=AF.Relu,
                )
                onl = out_nl[:, :, :, :].rearrange("t s p (b q) -> t s p b q", b=SB)
                for e in range(2):
                    ob = obpool.tile([128, SB * 511], f32, tag="obn", name="obn")
                    obv = ob[:].rearrange("p (b q) -> p b q", b=SB)
                    nc.scalar.activation(
                        out=obv, in_=hv[:, e, :, 0:511], func=AF.Relu
                    )
                    nc.sync.dma_start(out=onl[e, s, :, :, 0:511], in_=obv)

            # ---- pipeline ----------------------------------------------
            nc.sync.dma_start(out=idx[:], in_=idx_d[:, :])
            for s in range(NSTAGE):
                gb = [emit_gather(SB * s + bb) for bb in range(SB)]
                if s == 0:
                    nc.sync.dma_start(out=cpk[:], in_=cpk_d[:, :])
                    nc.vector.tensor_copy(out=wc[:], in_=cpk[:, 0:512])
                    nc.vector.tensor_copy(out=ws[:], in_=cpk[:, 512:1024])
                if s >= 1:
                    emit_tree_out(s - 1)
                for bb in range(SB):
                    emit_init_compute(s, bb, gb[bb])
                    emit_leaf_out(s, bb)
            emit_tree_out(NSTAGE - 1)
            nc.sync.dma_start(out=out_max[:, :], in_=mx[:])

    nc.compile()
    return nc


def kernel(**inputs):
    from concourse.bass_utils import run_bass_kernel_spmd

    encodes = np.ascontiguousarray(np.asarray(inputs["encodes"], dtype=np.float32))
    node_ids = np.asarray(inputs["node_ids"])
    wc = np.asarray(inputs["W_c_w"], dtype=np.float32).T  # [d, e]
    ws = np.asarray(inputs["W_sum_w"], dtype=np.float32).T
    bc = np.asarray(inputs["W_c_b"], dtype=np.float32)
    bs = np.asarray(inputs["W_sum_b"], dtype=np.float32)

    cpk = np.zeros((128, CPACK), np.float32)
    cpk[:, 0:256] = wc[0:128, :]
    cpk[:, 256:512] = wc[128:256, :]
    cpk[:, 512:768] = ws[0:128, :]
    cpk[:, 768:1024] = ws[128:256, :]
    cpk[:, 1024] = bc[0:128]
    cpk[:, 1025] = bc[128:256]
    cpk[:, 1026] = bc[0:128] + 2.0 * bs[0:128]
    cpk[:, 1027] = bc[128:256] + 2.0 * bs[128:256]
    cpk[:, 1028:1156] = np.eye(128, dtype=np.float32)

    in_maps = []
    for c in range(N_CORES):
        nid = np.asarray(node_ids[:, c * B_LOC : (c + 1) * B_LOC], dtype=np.int64)
        if GATHER == "dma_gather":
            cols = []
            for b in range(B_LOC):
                arr = np.zeros(NI, np.int64)
                arr[0:P] = nid[:, b] - HALF
                arr[P:] = 0  # dummies: offset 0, keep trailing idx >= 0
                wrapped = arr.astype(np.int16).reshape(NI // 16, 16).T
                cols.append(np.tile(wrapped, (8, 1)))
            idx = np.ascontiguousarray(np.concatenate(cols, axis=1))
        else:
            idx = np.zeros((128, 64), np.int32)
            for b in range(B_LOC):
                col = np.zeros(1024, np.int64)
                col[:P] = nid[:, b]
                idx[:, b * 8 : (b + 1) * 8] = col.reshape(8, 128).T
            idx = np.ascontiguousarray(idx)
        in_maps.append({"encodes": encodes, "idx": idx, "cpack": cpk})

    if _NC_CACHE[0] is None:
        _NC_CACHE[0] = _build_nc()
    nc = _NC_CACHE[0]

    res = run_bass_kernel_spmd(
        nc, in_maps, core_ids=list(range(N_CORES)), trace=TRACE[0]
    )
    LAST_RESULT[0] = res

    node_list = np.empty((P, B, E), np.float32)
    mx = np.empty((B, E), np.float32)
    for c in range(N_CORES):
        r = res.results[c]
        fm = r["out_nl"]  # [2, 4, 128, 2046]
        nl = (
            fm.reshape(2, NSTAGE, 128, SB, P)
            .transpose(1, 3, 4, 0, 2)
            .reshape(B_LOC, P, E)
        )
        node_list[:, c * B_LOC : (c + 1) * B_LOC, :] = nl.transpose(1, 0, 2)[POSTORDER]
        om = r["out_max"]
        mx[c * B_LOC : (c + 1) * B_LOC, 0:128] = om[:, 0:B_LOC].T
        mx[c * B_LOC : (c + 1) * B_LOC, 128:256] = om[:, B_LOC : 2 * B_LOC].T
    return node_list, mx


# revision 19
# speedup vs baseline: 1.1138x; 1.1138x over previous
"""Trainium2 Bass kernel for nn_BatchASTEncoder (batched AST / complete-binary-tree
GNN message passing).

Math (per batch column b):
    h[p] = W_c @ encodes[node_ids[p, b]] + b_c                    (1023 tree positions)
    for level d = 8..0:  h[parent] += W_sum @ (h[left] + h[right]) + 2*b_sum
    node_list = relu(h[POSTORDER]);  out2 = max_p node_list

Distribution: data-parallel over batch B=64 across 8 NeuronCores (8 columns
per core); encodes and the tiny weights are replicated.

Per-core dataflow (B_loc=8 batch columns, stage = 2 batch columns, stage-local
column i = bb*1023 + p):
  - h kept feature-major in SBUF: per (e-chunk, stage) tiles [128, 2046].
    Feature-major makes tree pair-sums strided free-dim vector ops and
    matmuls layout-preserving.
  - Software pipeline, one stage in flight per phase: the serial SWDGE gather
    descriptor generation paces the kernel; stage s-1's tree is emitted
    before stage s's PE work so it executes inside stage s's gather window.
  - Gather: one dma_gather per stage (2046 rows + 18 dummies). dma_gather
    indices are int16, so in_ap is based at the table midpoint and indices
    are signed offsets (idx - 32768); trailing dummies keep the ext-isa
    kernel's trailing-negative truncation away from real rows.
  - Matmuls run in float32r (single-pass fp32, ~1e-4 relative error) instead
    of float32 (two-pass); producers of matmul operands write f32r tiles.
  - 2*W_sum_b is folded into the init bias of non-leaf positions.
  - Leaf columns (p >= 511) are final right after init: relu'd and stored
    while the tree for the stage is still pending.
  - Output is written feature-major; the host unshard step transposes to the
    reference row-major [P, B, E] layout and applies the postorder permute.
"""

import numpy as np

DEPTH = 10
P = 2**DEPTH - 1          # 1023
B = 64
E = 256
N_TOTAL = B * P           # 65472
HALF = 32768              # dma_gather int16 base offset
N_CORES = 8
B_LOC = B // N_CORES      # 8
R = B_LOC * P             # 8184 columns per core
NSTAGE = 4
SB = 2                    # batch columns per stage
SW = SB * P               # stage width: 2046
NI = 1040                 # dma_gather indices per call (1 batch column + 17 dummies)
NIB = 9                   # dst j-blocks per call
GATHER = "dma_gather"     # or "indirect"
CPACK = 1156              # packed f32 consts: wc(512) ws(512) bias(4) ident(128)
USE_F32R = True


def _postorder(p, out):
    if p >= P:
        return
    _postorder(2 * p + 1, out)
    _postorder(2 * p + 2, out)
    out.append(p)


_PO = []
_postorder(0, _PO)
POSTORDER = np.array(_PO, dtype=np.int32)

_NC_CACHE = [None]
LAST_RESULT = [None]
TRACE = [False]


def _drain_runs(gi):
    """Per-b column runs for matmul group gi (p in [512gi, 512(gi+1)) cap 1023),
    split at the leaf boundary. Yields (p0, p1, leaf)."""
    runs = []
    p = 512 * gi
    end = min(512 * (gi + 1), P)
    while p < end:
        leaf = p >= 511
        p1 = min(end, P if leaf else 511)
        runs.append((p, p1, leaf))
        p = p1
    return runs


def _build_nc():
    import concourse.bacc as bacc
    import concourse.mybir as mybir
    import concourse.tile as tile
    from concourse import bass  # noqa: F401

    f32 = mybir.dt.float32
    f32m = mybir.dt.float32r if USE_F32R else f32
    i16 = mybir.dt.int16  # noqa
    AF = mybir.ActivationFunctionType
    AX = mybir.AxisListType

    nc = bacc.Bacc("TRN2", target_bir_lowering=False, debug=False)

    encodes = nc.dram_tensor("encodes", [N_TOTAL, E], f32, kind="ExternalInput")
    if GATHER == "dma_gather":
        idx_d = nc.dram_tensor(
            "idx", [128, B_LOC * (NI // 16)], i16, kind="ExternalInput"
        )
    else:
        idx_d = nc.dram_tensor("idx", [128, 64], mybir.dt.int32, kind="ExternalInput")
    cpk_d = nc.dram_tensor("cpack", [128, CPACK], f32, kind="ExternalInput")
    out_nl = nc.dram_tensor("out_nl", [2, NSTAGE, 128, SW], f32, kind="ExternalOutput")
    out_max = nc.dram_tensor("out_max", [128, 2 * B_LOC], f32, kind="ExternalOutput")

    with tile.TileContext(nc) as tc:
        with (
            tc.tile_pool(name="const", bufs=1) as cpool,
            tc.tile_pool(name="h", bufs=1) as hpool,
            tc.tile_pool(name="g", bufs=5) as gpool,
            tc.tile_pool(name="gtg", bufs=3) as gtgpool,
            tc.tile_pool(name="ks", bufs=2) as kspool,
            tc.tile_pool(name="ob", bufs=2) as obpool,
            tc.tile_pool(name="mx", bufs=1) as mxpool,
            tc.tile_pool(name="tp", bufs=2, space="PSUM") as tppool,
            tc.tile_pool(name="hp", bufs=1, space="PSUM") as hppool,
            tc.tile_pool(name="cst", bufs=1, space="PSUM") as cstpool,
        ):
            cpk = cpool.tile([128, CPACK], f32)
            wc = cpool.tile([128, 2 * E], f32m)   # [d-chunk part, dc*256 + e]
            ws = cpool.tile([128, 2 * E], f32m)
            if GATHER == "dma_gather":
                idx = cpool.tile([128, B_LOC * (NI // 16)], i16)
            else:
                idx = cpool.tile([128, 64], mybir.dt.int32)
            hts = [hpool.tile([128, 2 * SW], f32, name=f"hs{s}") for s in range(NSTAGE)]
            mx = mxpool.tile([128, 2 * B_LOC], f32)
            ident = cpk[:, 1028:1156]

            def emit_dummy_mm():
                dmm = tppool.tile([128, 512], f32, tag="tp", name="dmm")
                nc.tensor.matmul(
                    out=dmm[:], lhsT=wc[:, 0:128], rhs=wc[:, :],
                    start=True, stop=True, skip_group_check=True,
                )

            def emit_gather(b):
                g = gpool.tile([128, NIB * E], f32, name="g")
                if GATHER == "dma_gather":
                    nc.gpsimd.dma_gather(
                        out_ap=g[:].rearrange("p (j e) -> p j e", e=E),
                        in_ap=encodes[HALF:, :],
                        idxs_ap=idx[:, b * (NI // 16) : (b + 1) * (NI // 16)],
                        num_idxs=NI,
                        num_idxs_reg=NI,
                        elem_size=E,
                        single_packet=False,
                    )
                else:
                    for j in range(8):
                        nc.gpsimd.indirect_dma_start(
                            out=g[:, j * E : (j + 1) * E],
                            out_offset=None,
                            in_=encodes[:, :],
                            in_offset=bass.IndirectOffsetOnAxis(
                                ap=idx[:, b * 8 + j : b * 8 + j + 1], axis=0
                            ),
                        )
                return g

            def emit_init_compute(s, bb, g):
                gtgs = {}
                for gi in range(2):
                    gtg = gtgpool.tile([128, 4 * E], f32m, name="gtg")
                    gtgs[gi] = gtg
                    tpd = [
                        tppool.tile([128, 512], f32, tag="tp", name=f"tpd{dc}")
                        for dc in range(2)
                    ]
                    for k in range(4):
                        j = 4 * gi + k
                        nc.tensor.transpose(
                            out=tpd[0][:, k * 128 : k * 128 + 128],
                            in_=g[:, j * E : j * E + 128],
                            identity=ident,
                        )
                        nc.tensor.transpose(
                            out=tpd[1][:, k * 128 : k * 128 + 128],
                            in_=g[:, j * E + 128 : (j + 1) * E],
                            identity=ident,
                        )
                    nc.vector.tensor_copy(out=gtg[:, 0:512], in_=tpd[0][:])
                    nc.scalar.copy(out=gtg[:, 512:1024], in_=tpd[1][:])
                hp = {
                    (e, gi): hppool.tile(
                        [128, 512], f32, tag=f"hp{e}{gi}", name=f"hp{e}{gi}"
                    )
                    for e in range(2)
                    for gi in range(2)
                }
                # weights-outer: one LDWEIGHTS serves both groups
                for e in range(2):
                    for dc in range(2):
                        lhs = wc[:, 256 * dc + 128 * e : 256 * dc + 128 * e + 128]
                        for gi in range(2):
                            nc.tensor.matmul(
                                out=hp[(e, gi)][:],
                                lhsT=lhs,
                                rhs=gtgs[gi][:, 512 * dc : 512 * dc + 512],
                                start=(dc == 0),
                                stop=(dc == 1),
                            )
                emit_dummy_mm()
                for gi in range(2):
                    for e in range(2):
                        for p0, p1, leaf in _drain_runs(gi):
                            bcol = 1024 + (e if leaf else 2 + e)
                            nc.scalar.activation(
                                out=hts[s][
                                    :, e * SW + bb * P + p0 : e * SW + bb * P + p1
                                ],
                                in_=hp[(e, gi)][:, p0 - 512 * gi : p1 - 512 * gi],
                                func=AF.Identity,
                                bias=cpk[:, bcol : bcol + 1],
                                scale=1.0,
                            )

            def emit_leaf_out(s, bb):
                hv = hts[s][:, :].rearrange("p (e b q) -> p e b q", e=2, b=SB)
                ob = obpool.tile([128, 2 * 512], f32, tag="obl", name="obl")
                obv = ob[:].rearrange("p (e q) -> p e q", e=2)
                nc.scalar.activation(
                    out=obv, in_=hv[:, :, bb, 511:1023], func=AF.Relu
                )
                nc.sync.dma_start(
                    out=out_nl[:, s, :, bb * P + 511 : bb * P + 1023].transpose(
                        [1, 0, 2]
                    ),
                    in_=obv,
                )

            def emit_tree_out(s):
                hv = hts[s][:, :].rearrange("p (e b q) -> p e b q", e=2, b=SB)
                for d in range(DEPTH - 2, -1, -1):
                    p0 = 2**d - 1
                    n = 2**d
                    ks = kspool.tile([128, 2 * SB * n], f32m, tag="ks", name="ks")
                    kid2 = hv[:, :, :, 2 * p0 + 1 : 2 * p0 + 1 + 2 * n].rearrange(
                        "p e b (n two) -> p e b n two", two=2
                    )
                    nc.vector.tensor_add(
                        out=ks[:].rearrange("p (e b n) -> p e b n", e=2, b=SB),
                        in0=kid2[:, :, :, :, 0],
                        in1=kid2[:, :, :, :, 1],
                    )
                    ksv = ks[:].rearrange("p (e bn) -> p e bn", e=2)
                    cs = cstpool.tile([128, 1024], f32, tag="cst", name="cst")
                    for e in range(2):
                        nc.tensor.matmul(
                            out=cs[:, 512 * e : 512 * e + SB * n],
                            lhsT=ws[:, 128 * e : 128 * e + 128],
                            rhs=ksv[:, 0, :],
                            start=True, stop=False,
                        )
                        nc.tensor.matmul(
                            out=cs[:, 512 * e : 512 * e + SB * n],
                            lhsT=ws[:, 256 + 128 * e : 256 + 128 * e + 128],
                            rhs=ksv[:, 1, :],
                            start=False, stop=True,
                        )
                    nc.vector.tensor_add(
                        out=hv[:, :, :, p0 : p0 + n],
                        in0=hv[:, :, :, p0 : p0 + n],
                        in1=cs[:].rearrange("p (e bn) -> p e bn", e=2)[
                            :, :, : SB * n
                        ].rearrange("p e (b n) -> p e b n", b=SB),
                    )
                    emit_dummy_mm()
                # non-leaf columns now final: relu + store + stage max
                mxp = mxpool.tile([128, 2 * SB], f32, tag="mxp", name="mxp")
                nc.vector.tensor_reduce(
                    out=mxp[:], in_=hv[:, :, :, :], axis=AX.X,
                    op=mybir.AluOpType.max,
                )
                nc.scalar.activation(
                    out=mx[:, :].rearrange("p (e b) -> p e b", e=2)[
                        :, :, SB * s : SB * (s + 1)
                    ],
                    in_=mxp[:].rearrange("p (e b) -> p e b", e=2),
                    func=AF.Relu,
                )
                onl = out_nl[:, :, :, :].rearrange("t s p (b q) -> t s p b q", b=SB)
                for e in range(2):
                    ob = obpool.tile([128, SB * 511], f32, tag="obn", name="obn")
                    obv = ob[:].rearrange("p (b q) -> p b q", b=SB)
                    nc.scalar.activation(
                        out=obv, in_=hv[:, e, :, 0:511], func=AF.Relu
                    )
                    nc.sync.dma_start(out=onl[e, s, :, :, 0:511], in_=obv)

            # ---- pipeline ----------------------------------------------
            nc.sync.dma_start(out=idx[:], in_=idx_d[:, :])
            for s in range(NSTAGE):
                gb = [emit_gather(SB * s + bb) for bb in range(SB)]
                if s == 0:
                    nc.sync.dma_start(out=cpk[:], in_=cpk_d[:, :])
                    nc.vector.tensor_copy(out=wc[:], in_=cpk[:, 0:512])
                    nc.vector.tensor_copy(out=ws[:], in_=cpk[:, 512:1024])
                if s >= 1:
                    emit_tree_out(s - 1)
                for bb in range(SB):
                    emit_init_compute(s, bb, gb[bb])
                    emit_leaf_out(s, bb)
            emit_tree_out(NSTAGE - 1)
            nc.sync.dma_start(out=out_max[:, :], in_=mx[:])

    nc.compile()
    return nc


def kernel(**inputs):
    from concourse.bass_utils import run_bass_kernel_spmd

    encodes = np.ascontiguousarray(np.asarray(inputs["encodes"], dtype=np.float32))
    node_ids = np.asarray(inputs["node_ids"])
    wc = np.asarray(inputs["W_c_w"], dtype=np.float32).T  # [d, e]
    ws = np.asarray(inputs["W_sum_w"], dtype=np.float32).T
    bc = np.asarray(inputs["W_c_b"], dtype=np.float32)
    bs = np.asarray(inputs["W_sum_b"], dtype=np.float32)

    cpk = np.zeros((128, CPACK), np.float32)
    cpk[:, 0:256] = wc[0:128, :]
    cpk[:, 256:512] = wc[128:256, :]
    cpk[:, 512:768] = ws[0:128, :]
    cpk[:, 768:1024] = ws[128:256, :]
    cpk[:, 1024] = bc[0:128]
    cpk[:, 1025] = bc[128:256]
    cpk[:, 1026] = bc[0:128] + 2.0 * bs[0:128]
    cpk[:, 1027] = bc[128:256] + 2.0 * bs[128:256]
    cpk[:, 1028:1156] = np.eye(128, dtype=np.float32)

    in_maps = []
    for c in range(N_CORES):
        nid = np.asarray(node_ids[:, c * B_LOC : (c + 1) * B_LOC], dtype=np.int64)
        if GATHER == "dma_gather":
            cols = []
            for b in range(B_LOC):
                arr = np.zeros(NI, np.int64)
                arr[0:P] = nid[:, b] - HALF
                arr[P:] = 0  # dummies: offset 0, keep trailing idx >= 0
                wrapped = arr.astype(np.int16).reshape(NI // 16, 16).T
                cols.append(np.tile(wrapped, (8, 1)))
            idx = np.ascontiguousarray(np.concatenate(cols, axis=1))
        else:
            idx = np.zeros((128, 64), np.int32)
            for b in range(B_LOC):
                col = np.zeros(1024, np.int64)
                col[:P] = nid[:, b]
                idx[:, b * 8 : (b + 1) * 8] = col.reshape(8, 128).T
            idx = np.ascontiguousarray(idx)
        in_maps.append({"encodes": encodes, "idx": idx, "cpack": cpk})

    if _NC_CACHE[0] is None:
        _NC_CACHE[0] = _build_nc()
    nc = _NC_CACHE[0]

    res = run_bass_kernel_spmd(
        nc, in_maps, core_ids=list(range(N_CORES)), trace=TRACE[0]
    )
    LAST_RESULT[0] = res

    node_list = np.empty((P, B, E), np.float32)
    mx = np.empty((B, E), np.float32)
    for c in range(N_CORES):
        r = res.results[c]
        fm = r["out_nl"]  # [2, 4, 128, 2046]
        nl = (
            fm.reshape(2, NSTAGE, 128, SB, P)
            .transpose(1, 3, 4, 0, 2)
            .reshape(B_LOC, P, E)
        )
        node_list[:, c * B_LOC : (c + 1) * B_LOC, :] = nl.transpose(1, 0, 2)[POSTORDER]
        om = r["out_max"]
        mx[c * B_LOC : (c + 1) * B_LOC, 0:128] = om[:, 0:B_LOC].T
        mx[c * B_LOC : (c + 1) * B_LOC, 128:256] = om[:, B_LOC : 2 * B_LOC].T
    return node_list, mx


# revision 21
# speedup vs baseline: 1.1170x; 1.0029x over previous
"""Trainium2 Bass kernel for nn_BatchASTEncoder (batched AST / complete-binary-tree
GNN message passing).

Math (per batch column b):
    h[p] = W_c @ encodes[node_ids[p, b]] + b_c                    (1023 tree positions)
    for level d = 8..0:  h[parent] += W_sum @ (h[left] + h[right]) + 2*b_sum
    node_list = relu(h[POSTORDER]);  out2 = max_p node_list

Distribution: data-parallel over batch B=64 across 8 NeuronCores (8 columns
per core); encodes and the tiny weights are replicated.

Per-core dataflow (B_loc=8 batch columns, stage = 2 batch columns, stage-local
column i = bb*1023 + p):
  - h kept feature-major in SBUF: per (e-chunk, stage) tiles [128, 2046].
    Feature-major makes tree pair-sums strided free-dim vector ops and
    matmuls layout-preserving.
  - Software pipeline, one stage in flight per phase: the serial SWDGE gather
    descriptor generation paces the kernel; stage s-1's tree is emitted
    before stage s's PE work so it executes inside stage s's gather window.
  - Gather: one dma_gather per stage (2046 rows + 18 dummies). dma_gather
    indices are int16, so in_ap is based at the table midpoint and indices
    are signed offsets (idx - 32768); trailing dummies keep the ext-isa
    kernel's trailing-negative truncation away from real rows.
  - Matmuls run in float32r (single-pass fp32, ~1e-4 relative error) instead
    of float32 (two-pass); producers of matmul operands write f32r tiles.
  - 2*W_sum_b is folded into the init bias of non-leaf positions.
  - Leaf columns (p >= 511) are final right after init: relu'd and stored
    while the tree for the stage is still pending.
  - Output is written feature-major; the host unshard step transposes to the
    reference row-major [P, B, E] layout and applies the postorder permute.
"""

import numpy as np

DEPTH = 10
P = 2**DEPTH - 1          # 1023
B = 64
E = 256
N_TOTAL = B * P           # 65472
HALF = 32768              # dma_gather int16 base offset
N_CORES = 8
B_LOC = B // N_CORES      # 8
R = B_LOC * P             # 8184 columns per core
NSTAGE = 4
SB = 2                    # batch columns per stage
SW = SB * P               # stage width: 2046
NI = 1040                 # dma_gather indices per call (1 batch column + 17 dummies)
NIB = 9                   # dst j-blocks per call
GATHER = "dma_gather"     # or "indirect"
CPACK = 1156              # packed f32 consts: wc(512) ws(512) bias(4) ident(128)
USE_F32R = True


def _postorder(p, out):
    if p >= P:
        return
    _postorder(2 * p + 1, out)
    _postorder(2 * p + 2, out)
    out.append(p)


_PO = []
_postorder(0, _PO)
POSTORDER = np.array(_PO, dtype=np.int32)

_NC_CACHE = [None]
LAST_RESULT = [None]
TRACE = [False]


def _drain_runs(gi):
    """Per-b column runs for matmul group gi (p in [512gi, 512(gi+1)) cap 1023),
    split at the leaf boundary. Yields (p0, p1, leaf)."""
    runs = []
    p = 512 * gi
    end = min(512 * (gi + 1), P)
    while p < end:
        leaf = p >= 511
        p1 = min(end, P if leaf else 511)
        runs.append((p, p1, leaf))
        p = p1
    return runs


def _build_nc():
    import concourse.bacc as bacc
    import concourse.mybir as mybir
    import concourse.tile as tile
    from concourse import bass  # noqa: F401

    f32 = mybir.dt.float32
    f32m = mybir.dt.float32r if USE_F32R else f32
    i16 = mybir.dt.int16  # noqa
    AF = mybir.ActivationFunctionType
    AX = mybir.AxisListType

    nc = bacc.Bacc("TRN2", target_bir_lowering=False, debug=False)

    encodes = nc.dram_tensor("encodes", [N_TOTAL, E], f32, kind="ExternalInput")
    if GATHER == "dma_gather":
        idx_d = nc.dram_tensor(
            "idx", [128, B_LOC * (NI // 16)], i16, kind="ExternalInput"
        )
    else:
        idx_d = nc.dram_tensor("idx", [128, 64], mybir.dt.int32, kind="ExternalInput")
    cpk_d = nc.dram_tensor("cpack", [128, CPACK], f32, kind="ExternalInput")
    out_nl = nc.dram_tensor("out_nl", [2, NSTAGE, 128, SW], f32, kind="ExternalOutput")
    out_max = nc.dram_tensor("out_max", [128, 2 * B_LOC], f32, kind="ExternalOutput")

    with tile.TileContext(nc) as tc:
        with (
            tc.tile_pool(name="const", bufs=1) as cpool,
            tc.tile_pool(name="h", bufs=1) as hpool,
            tc.tile_pool(name="g", bufs=5) as gpool,
            tc.tile_pool(name="gtg", bufs=3) as gtgpool,
            tc.tile_pool(name="ks", bufs=2) as kspool,
            tc.tile_pool(name="ob", bufs=2) as obpool,
            tc.tile_pool(name="mx", bufs=1) as mxpool,
            tc.tile_pool(name="tp", bufs=2, space="PSUM") as tppool,
            tc.tile_pool(name="hp", bufs=1, space="PSUM") as hppool,
            tc.tile_pool(name="cst", bufs=1, space="PSUM") as cstpool,
        ):
            cpk = cpool.tile([128, CPACK], f32)
            wc = cpool.tile([128, 2 * E], f32m)   # [d-chunk part, dc*256 + e]
            ws = cpool.tile([128, 2 * E], f32m)
            if GATHER == "dma_gather":
                idx = cpool.tile([128, B_LOC * (NI // 16)], i16)
            else:
                idx = cpool.tile([128, 64], mybir.dt.int32)
            hts = [hpool.tile([128, 2 * SW], f32, name=f"hs{s}") for s in range(NSTAGE)]
            mx = mxpool.tile([128, 2 * B_LOC], f32)
            ident = cpk[:, 1028:1156]

            def emit_dummy_mm():
                dmm = tppool.tile([128, 512], f32, tag="tp", name="dmm")
                nc.tensor.matmul(
                    out=dmm[:], lhsT=wc[:, 0:128], rhs=wc[:, :],
                    start=True, stop=True, skip_group_check=True,
                )

            def emit_gather(b):
                g = gpool.tile([128, NIB * E], f32, name="g")
                if GATHER == "dma_gather":
                    nc.gpsimd.dma_gather(
                        out_ap=g[:].rearrange("p (j e) -> p j e", e=E),
                        in_ap=encodes[HALF:, :],
                        idxs_ap=idx[:, b * (NI // 16) : (b + 1) * (NI // 16)],
                        num_idxs=NI,
                        num_idxs_reg=NI,
                        elem_size=E,
                        single_packet=False,
                    )
                else:
                    for j in range(8):
                        nc.gpsimd.indirect_dma_start(
                            out=g[:, j * E : (j + 1) * E],
                            out_offset=None,
                            in_=encodes[:, :],
                            in_offset=bass.IndirectOffsetOnAxis(
                                ap=idx[:, b * 8 + j : b * 8 + j + 1], axis=0
                            ),
                        )
                return g

            def emit_init_compute(s, bb, g):
                # gtg: d-major transposed gather for one batch column
                # [128, 2048]: d-chunk 0 rows at cols 0:1024, chunk 1 at 1024:2048
                gtg = gtgpool.tile([128, 2048], f32m, name="gtg")
                for gi in range(2):
                    tpd = [
                        tppool.tile([128, 512], f32, tag="tp", name=f"tpd{dc}")
                        for dc in range(2)
                    ]
                    for k in range(4):
                        j = 4 * gi + k
                        nc.tensor.transpose(
                            out=tpd[0][:, k * 128 : k * 128 + 128],
                            in_=g[:, j * E : j * E + 128],
                            identity=ident,
                        )
                        nc.tensor.transpose(
                            out=tpd[1][:, k * 128 : k * 128 + 128],
                            in_=g[:, j * E + 128 : (j + 1) * E],
                            identity=ident,
                        )
                    nc.vector.tensor_copy(
                        out=gtg[:, 512 * gi : 512 * gi + 512], in_=tpd[0][:]
                    )
                    nc.scalar.copy(
                        out=gtg[:, 1024 + 512 * gi : 1536 + 512 * gi], in_=tpd[1][:]
                    )
                # matmul groups split at the leaf boundary: [0,511) / [511,1023)
                grps = ((0, 512, None), (512, 1024, None))
                hp = {
                    (e, gi): hppool.tile(
                        [128, 512], f32, tag=f"hp{e}{gi}", name=f"hp{e}{gi}"
                    )
                    for e in range(2)
                    for gi in range(2)
                }
                for e in range(2):
                    for dc in range(2):
                        lhs = wc[:, 256 * dc + 128 * e : 256 * dc + 128 * e + 128]
                        for gi, (p0, p1, leaf) in enumerate(grps):
                            nc.tensor.matmul(
                                out=hp[(e, gi)][:, : p1 - p0],
                                lhsT=lhs,
                                rhs=gtg[:, 1024 * dc + p0 : 1024 * dc + p1],
                                start=(dc == 0),
                                stop=(dc == 1),
                            )
                emit_dummy_mm()
                for gi in range(2):
                    for e in range(2):
                        for p0, p1, leaf in _drain_runs(gi):
                            bcol = 1024 + (e if leaf else 2 + e)
                            nc.scalar.activation(
                                out=hts[s][
                                    :, e * SW + bb * P + p0 : e * SW + bb * P + p1
                                ],
                                in_=hp[(e, gi)][:, p0 - 512 * gi : p1 - 512 * gi],
                                func=AF.Identity,
                                bias=cpk[:, bcol : bcol + 1],
                                scale=1.0,
                            )

            def emit_leaf_out(s, bb):
                hv = hts[s][:, :].rearrange("p (e b q) -> p e b q", e=2, b=SB)
                ob = obpool.tile([128, 2 * 512], f32, tag="obl", name="obl")
                obv = ob[:].rearrange("p (e q) -> p e q", e=2)
                nc.scalar.activation(
                    out=obv, in_=hv[:, :, bb, 511:1023], func=AF.Relu
                )
                nc.sync.dma_start(
                    out=out_nl[:, s, :, bb * P + 511 : bb * P + 1023].transpose(
                        [1, 0, 2]
                    ),
                    in_=obv,
                )

            def emit_tree_out(s):
                hv = hts[s][:, :].rearrange("p (e b q) -> p e b q", e=2, b=SB)
                for d in range(DEPTH - 2, -1, -1):
                    p0 = 2**d - 1
                    n = 2**d
                    ks = kspool.tile([128, 2 * SB * n], f32m, tag="ks", name="ks")
                    kid2 = hv[:, :, :, 2 * p0 + 1 : 2 * p0 + 1 + 2 * n].rearrange(
                        "p e b (n two) -> p e b n two", two=2
                    )
                    nc.vector.tensor_add(
                        out=ks[:].rearrange("p (e b n) -> p e b n", e=2, b=SB),
                        in0=kid2[:, :, :, :, 0],
                        in1=kid2[:, :, :, :, 1],
                    )
                    ksv = ks[:].rearrange("p (e bn) -> p e bn", e=2)
                    cs = cstpool.tile([128, 1024], f32, tag="cst", name="cst")
                    for e in range(2):
                        nc.tensor.matmul(
                            out=cs[:, 512 * e : 512 * e + SB * n],
                            lhsT=ws[:, 128 * e : 128 * e + 128],
                            rhs=ksv[:, 0, :],
                            start=True, stop=False,
                        )
                        nc.tensor.matmul(
                            out=cs[:, 512 * e : 512 * e + SB * n],
                            lhsT=ws[:, 256 + 128 * e : 256 + 128 * e + 128],
                            rhs=ksv[:, 1, :],
                            start=False, stop=True,
                        )
                    nc.vector.tensor_add(
                        out=hv[:, :, :, p0 : p0 + n],
                        in0=hv[:, :, :, p0 : p0 + n],
                        in1=cs[:].rearrange("p (e bn) -> p e bn", e=2)[
                            :, :, : SB * n
                        ].rearrange("p e (b n) -> p e b n", b=SB),
                    )
                    emit_dummy_mm()
                # non-leaf columns now final: relu + store + stage max
                mxp = mxpool.tile([128, 2 * SB], f32, tag="mxp", name="mxp")
                nc.vector.tensor_reduce(
                    out=mxp[:], in_=hv[:, :, :, :], axis=AX.X,
                    op=mybir.AluOpType.max,
                )
                nc.scalar.activation(
                    out=mx[:, :].rearrange("p (e b) -> p e b", e=2)[
                        :, :, SB * s : SB * (s + 1)
                    ],
                    in_=mxp[:].rearrange("p (e b) -> p e b", e=2),
                    func=AF.Relu,
                )
                onl = out_nl[:, :, :, :].rearrange("t s p (b q) -> t s p b q", b=SB)
                for e in range(2):
                    ob = obpool.tile([128, SB * 511], f32, tag="obn", name="obn")
                    obv = ob[:].rearrange("p (b q) -> p b q", b=SB)
                    nc.scalar.activation(
                        out=obv, in_=hv[:, e, :, 0:511], func=AF.Relu
                    )
                    nc.sync.dma_start(out=onl[e, s, :, :, 0:511], in_=obv)

            # ---- pipeline ----------------------------------------------
            nc.sync.dma_start(out=idx[:], in_=idx_d[:, :])
            for s in range(NSTAGE):
                gb = [emit_gather(SB * s + bb) for bb in range(SB)]
                if s == 0:
                    nc.sync.dma_start(out=cpk[:], in_=cpk_d[:, :])
                    nc.vector.tensor_copy(out=wc[:], in_=cpk[:, 0:512])
                    nc.vector.tensor_copy(out=ws[:], in_=cpk[:, 512:1024])
                if s >= 1:
                    emit_tree_out(s - 1)
                for bb in range(SB):
                    emit_init_compute(s, bb, gb[bb])
                    emit_leaf_out(s, bb)
            emit_tree_out(NSTAGE - 1)
            nc.sync.dma_start(out=out_max[:, :], in_=mx[:])

    nc.compile()
    return nc


def kernel(**inputs):
    from concourse.bass_utils import run_bass_kernel_spmd

    encodes = np.ascontiguousarray(np.asarray(inputs["encodes"], dtype=np.float32))
    node_ids = np.asarray(inputs["node_ids"])
    wc = np.asarray(inputs["W_c_w"], dtype=np.float32).T  # [d, e]
    ws = np.asarray(inputs["W_sum_w"], dtype=np.float32).T
    bc = np.asarray(inputs["W_c_b"], dtype=np.float32)
    bs = np.asarray(inputs["W_sum_b"], dtype=np.float32)

    cpk = np.zeros((128, CPACK), np.float32)
    cpk[:, 0:256] = wc[0:128, :]
    cpk[:, 256:512] = wc[128:256, :]
    cpk[:, 512:768] = ws[0:128, :]
    cpk[:, 768:1024] = ws[128:256, :]
    cpk[:, 1024] = bc[0:128]
    cpk[:, 1025] = bc[128:256]
    cpk[:, 1026] = bc[0:128] + 2.0 * bs[0:128]
    cpk[:, 1027] = bc[128:256] + 2.0 * bs[128:256]
    cpk[:, 1028:1156] = np.eye(128, dtype=np.float32)

    in_maps = []
    for c in range(N_CORES):
        nid = np.asarray(node_ids[:, c * B_LOC : (c + 1) * B_LOC], dtype=np.int64)
        if GATHER == "dma_gather":
            cols = []
            for b in range(B_LOC):
                arr = np.zeros(NI, np.int64)
                arr[0:P] = nid[:, b] - HALF
                arr[P:] = 0  # dummies: offset 0, keep trailing idx >= 0
                wrapped = arr.astype(np.int16).reshape(NI // 16, 16).T
                cols.append(np.tile(wrapped, (8, 1)))
            idx = np.ascontiguousarray(np.concatenate(cols, axis=1))
        else:
            idx = np.zeros((128, 64), np.int32)
            for b in range(B_LOC):
                col = np.zeros(1024, np.int64)
                col[:P] = nid[:, b]
                idx[:, b * 8 : (b + 1) * 8] = col.reshape(8, 128).T
            idx = np.ascontiguousarray(idx)
        in_maps.append({"encodes": encodes, "idx": idx, "cpack": cpk})

    if _NC_CACHE[0] is None:
        _NC_CACHE[0] = _build_nc()
    nc = _NC_CACHE[0]

    res = run_bass_kernel_spmd(
        nc, in_maps, core_ids=list(range(N_CORES)), trace=TRACE[0]
    )
    LAST_RESULT[0] = res

    node_list = np.empty((P, B, E), np.float32)
    mx = np.empty((B, E), np.float32)
    for c in range(N_CORES):
        r = res.results[c]
        fm = r["out_nl"]  # [2, 4, 128, 2046]
        nl = (
            fm.reshape(2, NSTAGE, 128, SB, P)
            .transpose(1, 3, 4, 0, 2)
            .reshape(B_LOC, P, E)
        )
        node_list[:, c * B_LOC : (c + 1) * B_LOC, :] = nl.transpose(1, 0, 2)[POSTORDER]
        om = r["out_max"]
        mx[c * B_LOC : (c + 1) * B_LOC, 0:128] = om[:, 0:B_LOC].T
        mx[c * B_LOC : (c + 1) * B_LOC, 128:256] = om[:, B_LOC : 2 * B_LOC].T
    return node_list, mx
